# revision 24
# baseline (speedup 1.0000x reference)
"""Trainium2 Bass kernel for nn_CamMemory (soft cross-entropy vs. memory bank).

Computes: x = normalize(inputs); logits = x @ features.T / TEMP;
loss = mean_b( lse(logits_b) - dot(softmax(targets_b), logits_b) )

Sharding: features/targets split row-wise (N dim) across 8 cores; inputs
replicated.  Each core returns partial stats (s, p, u) per batch row:
  s = sum_n exp(logits - SHIFT)      (partial sum-exp, fixed shift; |logits|<=21)
  p = sum_n exp(targets - 1)*logits  (partial weighted logit sum)
  u = sum_n exp(targets - 1)         (partial softmax denominator; targets in [0,1))
Host combines: loss = mean_b( SHIFT + log(sum s) - (sum p)/(sum u) ).

Per-core pipeline (DMA budget is the 16.8MB feature load; PE stream work is
kept at ~23us so it hides fully under the ~44us wire time):
  - SWDGE cast-DMA features f32 DRAM -> bf16 SBUF, natural layout (n on
    partitions), 1MB chunks, issued as the FIRST gpsimd work.
  - PE transpose-mode flips 128x128 blocks into PSUM (d on partitions);
    DVE copies batches of 8 blocks to SBUF featT.
  - Matmuls use featT blocks as STATIONARY and xT (64 cols) as MOVING ->
    logitsT [128n, 64b] in PSUM, 64 cycles each (half the moving cycles of
    the b-major orientation).
  - Epilogue per chunk with n on partitions: ACT exp -> el (bf16), DVE
    et*logits -> pm (bf16); s/p/u reduced over n by accumulating
    ones-matmuls into one PSUM bank ([1,192]).
"""

import numpy as np

import concourse.bacc as bacc
import concourse.mybir as mybir
import concourse.tile as tile
from concourse.masks import make_identity

B = 64
D = 2048
N = 16384
NUM_CORES = 8
NSH = N // NUM_CORES  # 2048 rows of features per core
TEMP = 0.05
SHIFT = 21.0  # |logits| <= (1/TEMP)*|x.f| <= 20*(1+eps) since both unit-norm

F32 = mybir.dt.float32
BF16 = mybir.dt.bfloat16


def build_nc(d=D, nsh=NSH, b=B, debug=False, no_epi=False, no_ones=False,
             sb_et=False, no_mm=False, no_xchain=False, no_warm=False,
             no_tr=False, no_small_tr=False):
    """Build the single-core Bass program (SPMD: same program, 8 shards)."""
    kc = d // 128     # contraction chunks (d on partitions)
    nch = nsh // 128  # feature-row chunks
    TG = 8            # transposed blocks staged per PSUM bank
    ngrp = kc // TG
    NWARM = 32

    nc = bacc.Bacc("TRN2", target_bir_lowering=False, debug=debug)

    inputs_d = nc.dram_tensor("inputs", [b, d], F32, kind="ExternalInput")
    targets_d = nc.dram_tensor("targets", [b, nsh], F32, kind="ExternalInput")
    features_d = nc.dram_tensor("features", [nsh, d], F32, kind="ExternalInput")
    out_d = nc.dram_tensor("out", [1, 192], F32, kind="ExternalOutput")

    with tile.TileContext(nc) as tc:
        with (
            tc.tile_pool(name="small", bufs=1) as small,
            tc.tile_pool(name="nat", bufs=6) as natp,
            tc.tile_pool(name="ft", bufs=3) as ftp,
            tc.tile_pool(name="epi", bufs=3) as epi,
            tc.tile_pool(name="tps", bufs=3, space="PSUM") as tpsp,
            tc.tile_pool(name="lps", bufs=3, space="PSUM") as lpsp,
            tc.tile_pool(name="spu", bufs=2, space="PSUM") as spup,
        ):
            # ---- feature cast-DMAs first: gpsimd's first instructions are
            # dma_starts so HBM streaming begins as early as possible.
            pieces = [(i * 128, 128) for i in range(nch - 1)]
            pieces += [((nch - 1) * 128, 64), ((nch - 1) * 128 + 64, 64)]
            natcs = []
            for i, (r0, nr) in enumerate(pieces):
                natc = natp.tile([128, d], BF16, tag="nat")
                nc.gpsimd.dma_start(natc[0:nr, :], features_d[r0:r0 + nr, :])
                natcs.append(natc)
                if i == 1:
                    # identity (gpsimd memset+affine) between the first two
                    # issues and the rest; needed by warmup/transposes ~7.5us.
                    ident = small.tile([128, 128], BF16)
                    make_identity(nc, ident[:])

            # constants (DVE memsets; do not touch gpsimd)
            bias_m1 = small.tile([128, 1], F32)
            nc.vector.memset(bias_m1[:], -1.0)
            bias_shift = small.tile([128, 1], F32)
            nc.vector.memset(bias_shift[:], -float(SHIFT))
            ones = small.tile([128, 1], BF16)
            nc.vector.memset(ones[:], 1.0)

            # HAM pre-warm: throwaway matmuls while the first cast-DMA is in
            # flight, so the PE clock gate is 8/8 (2.4GHz) for the real work.
            if not no_warm:
                dwarm = lpsp.tile([128, 64], F32, tag="lp")
                for _ in range(NWARM):
                    nc.tensor.matmul(dwarm[:], ident[:], ident[:, 0:64],
                                     start=True, stop=True)

            # ---- x chain: xn = (inputs/||inputs||)/TEMP as bf16, transposed
            if no_xchain:
                no_mm = True
            xT = small.tile([128, kc, 64], BF16)
            if not no_xchain:
                xin = small.tile([b, d], F32)
                nc.sync.dma_start(xin[:], inputs_d[:])
                sq = small.tile([b, d], F32)
                ss = small.tile([b, 1], F32)
                nc.scalar.activation(
                    sq[:], xin[:], mybir.ActivationFunctionType.Square,
                    accum_out=ss[:],
                )
                srt = small.tile([b, 1], F32)
                nc.scalar.activation(
                    srt[:], ss[:], mybir.ActivationFunctionType.Sqrt,
                    scale=float(TEMP) * float(TEMP),
                )
                inv = small.tile([b, 1], F32)
                nc.vector.reciprocal(inv[:], srt[:])
                xnb = small.tile([b, d], BF16)
                nc.vector.tensor_scalar_mul(xnb[:], xin[:], inv[:])
                # transpose x: 16 blocks [64,128] -> [128,64]
                for g in range(0 if no_small_tr else ngrp):
                    tpx = tpsp.tile([128, TG, 128], BF16, tag="tps")
                    for j in range(TG):
                        k = g * TG + j
                        nc.tensor.transpose(
                            tpx[:, j, 0:b], xnb[:, k * 128:(k + 1) * 128],
                            ident[0:b, 0:b])
                    nc.vector.tensor_copy(xT[:, g * TG:(g + 1) * TG, :],
                                          tpx[:, :, 0:b])

            # ---- targets chain: etT = exp(targets - 1), n on partitions
            etT = small.tile([128, nch, b], BF16)
            if not no_xchain:
                tg = small.tile([b, nsh], F32)
                nc.sync.dma_start(tg[:], targets_d[:])
                tgb = small.tile([b, nsh], BF16)
                nc.vector.tensor_copy(tgb[:], tg[:])
                for g in range(0 if no_small_tr else (nch // TG)):
                    tpt = tpsp.tile([128, TG, 128], BF16, tag="tps")
                    for j in range(TG):
                        c = g * TG + j
                        nc.tensor.transpose(
                            tpt[:, j, 0:b], tgb[:, c * 128:(c + 1) * 128],
                            ident[0:b, 0:b])
                    if sb_et:
                        ttmp = small.tile([128, TG, 64], BF16, tag="ttmp")
                        nc.vector.tensor_copy(ttmp[:], tpt[:, :, 0:b])
                        nc.scalar.activation(
                            etT[:, g * TG:(g + 1) * TG, :], ttmp[:],
                            mybir.ActivationFunctionType.Exp, bias=bias_m1[:])
                    else:
                        # exp(t-1) fused with the PSUM->SBUF move on ACT
                        nc.scalar.activation(
                            etT[:, g * TG:(g + 1) * TG, :], tpt[:, :, 0:b],
                            mybir.ActivationFunctionType.Exp, bias=bias_m1[:])

            # ---- s/p/u partials: per-chunk ones-matmul [1,192] (contiguous
            # start/stop groups), accumulated on DVE into SBUF.
            acc = small.tile([1, 192], F32)
            nc.vector.memset(acc[:], 0.0)

            def emit_epi(prev):
                if no_epi:
                    return
                r0, nr, plps = prev
                ci, po = r0 // 128, r0 % 128
                pe = po + nr
                ets = etT[po:pe, ci, :]
                el = epi.tile([128, 64], BF16, tag="el")
                nc.scalar.activation(
                    el[po:pe, :], plps[po:pe, :],
                    mybir.ActivationFunctionType.Exp, bias=bias_shift[po:pe])
                pm = epi.tile([128, 64], BF16, tag="pm")
                nc.vector.tensor_mul(pm[po:pe, :], ets, plps[po:pe, :])
                if no_ones:
                    return
                spu = spup.tile([1, 192], F32, tag="spu")
                nc.tensor.matmul(spu[:, 0:64], ones[po:pe, :], el[po:pe, :],
                                 start=True, stop=True)
                nc.tensor.matmul(spu[:, 64:128], ones[po:pe, :], pm[po:pe, :],
                                 start=True, stop=True)
                nc.tensor.matmul(spu[:, 128:192], ones[po:pe, :], ets,
                                 start=True, stop=True)
                nc.vector.tensor_add(acc[:], acc[:], spu[:])

            # ---- feature pipeline: per piece (128 rows; last 2 are 64-row
            # pieces to halve the end-of-kernel drain chain), transposes feed
            # featT; logits matmuls use featT blocks stationary, xT moving.
            # Copies alternate DVE/ACT so groups move in parallel; the
            # epilogue of piece i-1 is emitted between piece i's transposes
            # and matmuls so the PE never waits on ACT/DVE.
            prev = None
            for (r0, nr), natc in zip(pieces, natcs, strict=True):
                ftc = ftp.tile([128, kc, 128], BF16, tag="ftc")
                for g in range(ngrp):
                    tp = tpsp.tile([128, TG, 128], BF16, tag="tps")
                    for j in range(TG):
                        k = g * TG + j
                        nc.tensor.transpose(
                            tp[:, j, 0:nr], natc[0:nr, k * 128:(k + 1) * 128],
                            ident[0:nr, 0:nr])
                    dst = ftc[:, g * TG:(g + 1) * TG, 0:nr]
                    if g % 2 == 0:
                        nc.vector.tensor_copy(dst, tp[:, :, 0:nr])
                    else:
                        nc.scalar.copy(dst, tp[:, :, 0:nr])

                if prev is not None:
                    emit_epi(prev)
                if not no_mm:
                    po = r0 % 128
                    lps = lpsp.tile([128, 64], F32, tag="lp")
                    for k in range(kc):
                        nc.tensor.matmul(
                            lps[po:po + nr, :], ftc[:, k, 0:nr], xT[:, k, :],
                            start=(k == 0), stop=(k == kc - 1),
                        )
                    prev = (r0, nr, lps)
            if prev is not None:
                emit_epi(prev)

            # ---- output
            nc.sync.dma_start(out_d[:], acc[:])

    nc.compile()
    return nc


_NC_CACHE = None


def _run(inputs, trace=False, **spmd_kwargs):
    global _NC_CACHE
    from concourse.bass_utils import run_bass_kernel_spmd

    x = np.ascontiguousarray(np.asarray(inputs["inputs"], dtype=np.float32))
    t = np.asarray(inputs["targets"], dtype=np.float32)
    f = np.asarray(inputs["features"], dtype=np.float32)
    # cid is unused by the reference computation.

    if _NC_CACHE is None:
        _NC_CACHE = build_nc(debug=False)
    nc = _NC_CACHE

    in_maps = []
    for c in range(NUM_CORES):
        in_maps.append({
            "inputs": x,
            "targets": np.ascontiguousarray(t[:, c * NSH:(c + 1) * NSH]),
            "features": np.ascontiguousarray(f[c * NSH:(c + 1) * NSH, :]),
        })

    res = run_bass_kernel_spmd(
        nc, in_maps, core_ids=list(range(NUM_CORES)), trace=trace, **spmd_kwargs)
    outs = np.stack([r["out"] for r in res.results])  # [8, 1, 192]

    outs64 = outs.astype(np.float64).reshape(NUM_CORES, 192)
    s = outs64[:, 0:64].sum(0)
    p = outs64[:, 64:128].sum(0)
    u = outs64[:, 128:192].sum(0)
    lse = SHIFT + np.log(s)
    loss = np.mean(lse - p / u)
    return np.float32(loss), res


def kernel(**inputs: np.ndarray) -> np.ndarray:
    loss, _ = _run(inputs)
    return np.asarray(loss, dtype=np.float32)


# revision 25
# speedup vs baseline: 1.0433x; 1.0433x over previous
"""Trainium2 Bass kernel for nn_CamMemory (soft cross-entropy vs. memory bank).

Computes: x = normalize(inputs); logits = x @ features.T / TEMP;
loss = mean_b( lse(logits_b) - dot(softmax(targets_b), logits_b) )

Sharding: features/targets split row-wise (N dim) across 8 cores; inputs
replicated.  Each core returns partial stats (s, p, u) per batch row:
  s = sum_n exp(logits - SHIFT)      (partial sum-exp, fixed shift; |logits|<=21)
  p = sum_n exp(targets - 1)*logits  (partial weighted logit sum)
  u = sum_n exp(targets - 1)         (partial softmax denominator; targets in [0,1))
Host combines: loss = mean_b( SHIFT + log(sum s) - (sum p)/(sum u) ).

Per-core pipeline (DMA budget is the 16.8MB feature load; PE stream work is
kept at ~23us so it hides fully under the ~44us wire time):
  - SWDGE cast-DMA features f32 DRAM -> bf16 SBUF, natural layout (n on
    partitions), 1MB chunks, issued as the FIRST gpsimd work.
  - PE transpose-mode flips 128x128 blocks into PSUM (d on partitions);
    DVE copies batches of 8 blocks to SBUF featT.
  - Matmuls use featT blocks as STATIONARY and xT (64 cols) as MOVING ->
    logitsT [128n, 64b] in PSUM, 64 cycles each (half the moving cycles of
    the b-major orientation).
  - Epilogue per chunk with n on partitions: ACT exp -> el (bf16), DVE
    et*logits -> pm (bf16); s/p/u reduced over n by accumulating
    ones-matmuls into one PSUM bank ([1,192]).
"""

import numpy as np

import concourse.bacc as bacc
import concourse.mybir as mybir
import concourse.tile as tile
from concourse.masks import make_identity

B = 64
D = 2048
N = 16384
NUM_CORES = 8
NSH = N // NUM_CORES  # 2048 rows of features per core
TEMP = 0.05
SHIFT = 21.0  # |logits| <= (1/TEMP)*|x.f| <= 20*(1+eps) since both unit-norm

F32 = mybir.dt.float32
BF16 = mybir.dt.bfloat16


def build_nc(d=D, nsh=NSH, b=B, debug=False, no_epi=False, no_ones=False,
             sb_et=False, no_mm=False, no_xchain=False, no_warm=False,
             no_tr=False, no_small_tr=False):
    """Build the single-core Bass program (SPMD: same program, 8 shards)."""
    kc = d // 128     # contraction chunks (d on partitions)
    nch = nsh // 128  # feature-row chunks
    TG = 8            # transposed blocks staged per PSUM bank
    ngrp = kc // TG
    NWARM = 32

    nc = bacc.Bacc("TRN2", target_bir_lowering=False, debug=debug)

    inputs_d = nc.dram_tensor("inputs", [b, d], F32, kind="ExternalInput")
    targets_d = nc.dram_tensor("targets", [b, nsh], F32, kind="ExternalInput")
    features_d = nc.dram_tensor("features", [nsh, d], F32, kind="ExternalInput")
    out_d = nc.dram_tensor("out", [1, 192], F32, kind="ExternalOutput")

    with tile.TileContext(nc) as tc:
        with (
            tc.tile_pool(name="small", bufs=1) as small,
            tc.tile_pool(name="nat", bufs=6) as natp,
            tc.tile_pool(name="ft", bufs=3) as ftp,
            tc.tile_pool(name="epi", bufs=3) as epi,
            tc.tile_pool(name="tps", bufs=3, space="PSUM") as tpsp,
            tc.tile_pool(name="lps", bufs=3, space="PSUM") as lpsp,
            tc.tile_pool(name="spu", bufs=2, space="PSUM") as spup,
        ):
            # ---- feature cast-DMAs first: gpsimd's first instructions are
            # dma_starts so HBM streaming begins as early as possible.
            pieces = [(i * 128, 128) for i in range(nch - 1)]
            pieces += [((nch - 1) * 128, 64), ((nch - 1) * 128 + 64, 64)]
            natcs = []
            for i, (r0, nr) in enumerate(pieces):
                natc = natp.tile([128, d], BF16, tag="nat")
                nc.gpsimd.dma_start(natc[0:nr, :], features_d[r0:r0 + nr, :])
                natcs.append(natc)
                if i == 1:
                    # identity (gpsimd memset+affine) between the first two
                    # issues and the rest; needed by warmup/transposes ~7.5us.
                    ident = small.tile([128, 128], BF16)
                    make_identity(nc, ident[:])

            # constants (DVE memsets; do not touch gpsimd)
            bias_m1 = small.tile([128, 1], F32)
            nc.vector.memset(bias_m1[:], -1.0)
            bias_shift = small.tile([128, 1], F32)
            nc.vector.memset(bias_shift[:], -float(SHIFT))
            ones = small.tile([128, 1], BF16)
            nc.vector.memset(ones[:], 1.0)

            # HAM pre-warm: throwaway matmuls while the first cast-DMA is in
            # flight, so the PE clock gate is 8/8 (2.4GHz) for the real work.
            if not no_warm:
                dwarm = lpsp.tile([128, 64], F32, tag="lp")
                for _ in range(NWARM):
                    nc.tensor.matmul(dwarm[:], ident[:], ident[:, 0:64],
                                     start=True, stop=True)

            # ---- x chain: xn = (inputs/||inputs||)/TEMP as bf16, transposed
            if no_xchain:
                no_mm = True
            xT = small.tile([128, kc, 64], BF16)
            if not no_xchain:
                xin = small.tile([b, d], F32)
                nc.sync.dma_start(xin[:], inputs_d[:])
                sq = small.tile([b, d], F32)
                ss = small.tile([b, 1], F32)
                nc.scalar.activation(
                    sq[:], xin[:], mybir.ActivationFunctionType.Square,
                    accum_out=ss[:],
                )
                srt = small.tile([b, 1], F32)
                nc.scalar.activation(
                    srt[:], ss[:], mybir.ActivationFunctionType.Sqrt,
                    scale=float(TEMP) * float(TEMP),
                )
                inv = small.tile([b, 1], F32)
                nc.vector.reciprocal(inv[:], srt[:])
                xnb = small.tile([b, d], BF16)
                nc.vector.tensor_scalar_mul(xnb[:], xin[:], inv[:])
                # transpose x: 16 blocks [64,128] -> [128,64]
                for g in range(0 if no_small_tr else ngrp):
                    tpx = tpsp.tile([128, TG, 128], BF16, tag="tps")
                    for j in range(TG):
                        k = g * TG + j
                        nc.tensor.transpose(
                            tpx[:, j, 0:b], xnb[:, k * 128:(k + 1) * 128],
                            ident[0:b, 0:b])
                    nc.vector.tensor_copy(xT[:, g * TG:(g + 1) * TG, :],
                                          tpx[:, :, 0:b])

            # ---- targets chain: etT = exp(targets - 1), n on partitions
            etT = small.tile([128, nch, b], BF16)
            if not no_xchain:
                tg = small.tile([b, nsh], F32)
                nc.sync.dma_start(tg[:], targets_d[:])
                tgb = small.tile([b, nsh], BF16)
                nc.vector.tensor_copy(tgb[:], tg[:])
                for g in range(0 if no_small_tr else (nch // TG)):
                    tpt = tpsp.tile([128, TG, 128], BF16, tag="tps")
                    for j in range(TG):
                        c = g * TG + j
                        nc.tensor.transpose(
                            tpt[:, j, 0:b], tgb[:, c * 128:(c + 1) * 128],
                            ident[0:b, 0:b])
                    if sb_et:
                        ttmp = small.tile([128, TG, 64], BF16, tag="ttmp")
                        nc.vector.tensor_copy(ttmp[:], tpt[:, :, 0:b])
                        nc.scalar.activation(
                            etT[:, g * TG:(g + 1) * TG, :], ttmp[:],
                            mybir.ActivationFunctionType.Exp, bias=bias_m1[:])
                    else:
                        # exp(t-1) fused with the PSUM->SBUF move on ACT
                        nc.scalar.activation(
                            etT[:, g * TG:(g + 1) * TG, :], tpt[:, :, 0:b],
                            mybir.ActivationFunctionType.Exp, bias=bias_m1[:])

            # ---- s/p/u partials: per-chunk ones-matmul [1,192] (contiguous
            # start/stop groups), accumulated on DVE into SBUF.
            acc = small.tile([1, 192], F32)
            nc.vector.memset(acc[:], 0.0)

            def emit_epi(prev):
                if no_epi:
                    return
                r0, nr, plps = prev
                ci, po = r0 // 128, r0 % 128
                pe = po + nr
                ets = etT[po:pe, ci, :]
                el = epi.tile([128, 64], BF16, tag="el")
                nc.scalar.activation(
                    el[po:pe, :], plps[po:pe, :],
                    mybir.ActivationFunctionType.Exp, bias=bias_shift[po:pe])
                pm = epi.tile([128, 64], BF16, tag="pm")
                nc.vector.tensor_mul(pm[po:pe, :], ets, plps[po:pe, :])
                if no_ones:
                    return
                spu = spup.tile([1, 192], F32, tag="spu")
                nc.tensor.matmul(spu[:, 0:64], ones[po:pe, :], el[po:pe, :],
                                 start=True, stop=True)
                nc.tensor.matmul(spu[:, 64:128], ones[po:pe, :], pm[po:pe, :],
                                 start=True, stop=True)
                nc.tensor.matmul(spu[:, 128:192], ones[po:pe, :], ets,
                                 start=True, stop=True)
                nc.vector.tensor_add(acc[:], acc[:], spu[:])

            # ---- feature pipeline: per piece (128 rows; last 2 are 64-row
            # pieces to halve the end-of-kernel drain chain), transposes feed
            # featT; logits matmuls use featT blocks stationary, xT moving.
            # Copies alternate DVE/ACT so groups move in parallel; the
            # epilogue of piece i-1 is emitted between piece i's transposes
            # and matmuls so the PE never waits on ACT/DVE.
            prev = None
            for (r0, nr), natc in zip(pieces, natcs, strict=True):
                ftc = ftp.tile([128, kc, 128], BF16, tag="ftc")
                for g in range(ngrp):
                    tp = tpsp.tile([128, TG, 128], BF16, tag="tps")
                    for j in range(TG):
                        k = g * TG + j
                        nc.tensor.transpose(
                            tp[:, j, 0:nr], natc[0:nr, k * 128:(k + 1) * 128],
                            ident[0:nr, 0:nr])
                    dst = ftc[:, g * TG:(g + 1) * TG, 0:nr]
                    nc.vector.tensor_copy(dst, tp[:, :, 0:nr])

                if prev is not None:
                    emit_epi(prev)
                if not no_mm:
                    po = r0 % 128
                    lps = lpsp.tile([128, 64], F32, tag="lp")
                    for k in range(kc):
                        nc.tensor.matmul(
                            lps[po:po + nr, :], ftc[:, k, 0:nr], xT[:, k, :],
                            start=(k == 0), stop=(k == kc - 1),
                        )
                    prev = (r0, nr, lps)
            if prev is not None:
                emit_epi(prev)

            # ---- output
            nc.sync.dma_start(out_d[:], acc[:])

    nc.compile()
    return nc


_NC_CACHE = None


def _run(inputs, trace=False, **spmd_kwargs):
    global _NC_CACHE
    from concourse.bass_utils import run_bass_kernel_spmd

    x = np.ascontiguousarray(np.asarray(inputs["inputs"], dtype=np.float32))
    t = np.asarray(inputs["targets"], dtype=np.float32)
    f = np.asarray(inputs["features"], dtype=np.float32)
    # cid is unused by the reference computation.

    if _NC_CACHE is None:
        _NC_CACHE = build_nc(debug=False)
    nc = _NC_CACHE

    in_maps = []
    for c in range(NUM_CORES):
        in_maps.append({
            "inputs": x,
            "targets": np.ascontiguousarray(t[:, c * NSH:(c + 1) * NSH]),
            "features": np.ascontiguousarray(f[c * NSH:(c + 1) * NSH, :]),
        })

    res = run_bass_kernel_spmd(
        nc, in_maps, core_ids=list(range(NUM_CORES)), trace=trace, **spmd_kwargs)
    outs = np.stack([r["out"] for r in res.results])  # [8, 1, 192]

    outs64 = outs.astype(np.float64).reshape(NUM_CORES, 192)
    s = outs64[:, 0:64].sum(0)
    p = outs64[:, 64:128].sum(0)
    u = outs64[:, 128:192].sum(0)
    lse = SHIFT + np.log(s)
    loss = np.mean(lse - p / u)
    return np.float32(loss), res


def kernel(**inputs: np.ndarray) -> np.ndarray:
    loss, _ = _run(inputs)
    return np.asarray(loss, dtype=np.float32)


# revision 26
# speedup vs baseline: 1.0483x; 1.0048x over previous
"""Trainium2 Bass kernel for nn_CamMemory (soft cross-entropy vs. memory bank).

Computes: x = normalize(inputs); logits = x @ features.T / TEMP;
loss = mean_b( lse(logits_b) - dot(softmax(targets_b), logits_b) )

Sharding: features/targets split row-wise (N dim) across 8 cores; inputs
replicated.  Each core returns partial stats (s, p, u) per batch row:
  s = sum_n exp(logits - SHIFT)      (partial sum-exp, fixed shift; |logits|<=21)
  p = sum_n exp(targets - 1)*logits  (partial weighted logit sum)
  u = sum_n exp(targets - 1)         (partial softmax denominator; targets in [0,1))
Host combines: loss = mean_b( SHIFT + log(sum s) - (sum p)/(sum u) ).

Per-core pipeline (everything rides the one SWDGE queue at HBM line rate;
PE/DVE/ACT work is sized to hide fully under the ~47us wire time):
  - SWDGE cast-DMAs f32 DRAM -> bf16 SBUF: inputs (xb), targets (tgb), then
    the 17 feature pieces (15x128 rows + 2x64 rows; the short tail pieces
    halve the end-of-kernel drain chain).
  - x chain off the wire early: row norms via DVE (bf16 square + reduce),
    ACT sqrt, DVE reciprocal; xb2 = xb * 1/(T*||x||) then PE-transposed to
    xT [128d, 16, 64b] (so only Sqrt+Exp ACT tables load, no Square).
  - Feature pieces: PE transpose-mode flips 128x128 blocks into PSUM, DVE
    copies 8-block groups to SBUF featT; matmuls use featT blocks as
    STATIONARY and xT (64 cols) as MOVING -> logitsT [128n, 64b] in PSUM.
  - Epilogue per piece with n on partitions: ACT exp -> el (bf16), DVE
    etT*logits -> pm (bf16); s/p/u reduced over n with ones-matmuls into a
    [1,192] PSUM tile, DVE-accumulated in SBUF.
"""

import numpy as np

import concourse.bacc as bacc
import concourse.mybir as mybir
import concourse.tile as tile
from concourse.masks import make_identity

B = 64
D = 2048
N = 16384
NUM_CORES = 8
NSH = N // NUM_CORES  # 2048 rows of features per core
TEMP = 0.05
SHIFT = 21.0  # |logits| <= (1/TEMP)*|x.f| <= 20*(1+eps) since both unit-norm

F32 = mybir.dt.float32
BF16 = mybir.dt.bfloat16


def build_nc(d=D, nsh=NSH, b=B, debug=False):
    """Build the single-core Bass program (SPMD: same program, 8 shards)."""
    kc = d // 128     # contraction chunks (d on partitions)
    nch = nsh // 128  # feature-row chunks
    TG = 8            # transposed blocks staged per PSUM bank
    ngrp = kc // TG
    NWARM = 32

    nc = bacc.Bacc("TRN2", target_bir_lowering=False, debug=debug)

    inputs_d = nc.dram_tensor("inputs", [b, d], F32, kind="ExternalInput")
    targets_d = nc.dram_tensor("targets", [b, nsh], F32, kind="ExternalInput")
    features_d = nc.dram_tensor("features", [nsh, d], F32, kind="ExternalInput")
    out_d = nc.dram_tensor("out", [1, 192], F32, kind="ExternalOutput")

    with tile.TileContext(nc) as tc:
        with (
            tc.tile_pool(name="small", bufs=1) as small,
            tc.tile_pool(name="nat", bufs=8) as natp,
            tc.tile_pool(name="ft", bufs=3) as ftp,
            tc.tile_pool(name="epi", bufs=3) as epi,
            tc.tile_pool(name="tps", bufs=3, space="PSUM") as tpsp,
            tc.tile_pool(name="lps", bufs=3, space="PSUM") as lpsp,
            tc.tile_pool(name="spu", bufs=2, space="PSUM") as spup,
        ):
            # ---- SWDGE cast-DMAs first: xb, tgb, then feature pieces.
            xb = small.tile([b, d], BF16)
            nc.gpsimd.dma_start(xb[:], inputs_d[:])
            tgb = small.tile([b, nsh], BF16)
            nc.gpsimd.dma_start(tgb[:], targets_d[:])

            pieces = [(i * 128, 128) for i in range(nch - 1)]
            pieces += [((nch - 1) * 128, 64), ((nch - 1) * 128 + 64, 64)]
            natcs = []
            for i, (r0, nr) in enumerate(pieces):
                natc = natp.tile([128, d], BF16, tag="nat")
                nc.gpsimd.dma_start(natc[0:nr, :], features_d[r0:r0 + nr, :])
                natcs.append(natc)
                if i == 1:
                    # identity (gpsimd memset+affine) amid the issue stream;
                    # ready ~9us, first needed by the x transposes ~12us.
                    ident = small.tile([128, 128], BF16)
                    make_identity(nc, ident[:])

            # constants (DVE memsets; do not touch gpsimd)
            bias_m1 = small.tile([128, 1], F32)
            nc.vector.memset(bias_m1[:], -1.0)
            bias_shift = small.tile([128, 1], F32)
            nc.vector.memset(bias_shift[:], -float(SHIFT))
            ones = small.tile([128, 1], BF16)
            nc.vector.memset(ones[:], 1.0)
            wones = small.tile([128, 64], BF16)
            nc.vector.memset(wones[:], 1.0)

            # HAM pre-warm: throwaway matmuls while the first cast-DMAs are
            # in flight, so the PE clock gate is 8/8 for the real work.
            dwarm = lpsp.tile([128, 64], F32, tag="lp")
            for _ in range(NWARM):
                nc.tensor.matmul(dwarm[0:64, :], wones[:, 0:64], wones[:],
                                 start=True, stop=True)

            # ---- x chain: xb2 = xb/(T*||xb||), transposed to [128d, kc, 64b]
            sqb = small.tile([b, d], BF16)
            nc.vector.tensor_mul(sqb[:], xb[:], xb[:])
            ss = small.tile([b, 1], F32)
            nc.vector.reduce_sum(ss[:], sqb[:], axis=mybir.AxisListType.X)
            srt = small.tile([b, 1], F32)
            nc.scalar.activation(
                srt[:], ss[:], mybir.ActivationFunctionType.Sqrt,
                scale=float(TEMP) * float(TEMP),
            )
            inv = small.tile([b, 1], F32)
            nc.vector.reciprocal(inv[:], srt[:])
            xb2 = small.tile([b, d], BF16)
            nc.vector.tensor_scalar_mul(xb2[:], xb[:], inv[:])
            xT = small.tile([128, kc, 64], BF16)
            for g in range(ngrp):
                tpx = tpsp.tile([128, TG, 128], BF16, tag="tps")
                for j in range(TG):
                    k = g * TG + j
                    nc.tensor.transpose(
                        tpx[:, j, 0:b], xb2[:, k * 128:(k + 1) * 128],
                        ident[0:b, 0:b])
                nc.vector.tensor_copy(xT[:, g * TG:(g + 1) * TG, :],
                                      tpx[:, :, 0:b])

            # ---- targets chain: etT = exp(targets - 1), n on partitions
            etT = small.tile([128, nch, b], BF16)
            for g in range(nch // TG):
                tpt = tpsp.tile([128, TG, 128], BF16, tag="tps")
                for j in range(TG):
                    c = g * TG + j
                    nc.tensor.transpose(
                        tpt[:, j, 0:b], tgb[:, c * 128:(c + 1) * 128],
                        ident[0:b, 0:b])
                # exp(t-1) fused with the PSUM->SBUF move on ACT
                nc.scalar.activation(
                    etT[:, g * TG:(g + 1) * TG, :], tpt[:, :, 0:b],
                    mybir.ActivationFunctionType.Exp, bias=bias_m1[:])

            # ---- s/p/u partials: per-piece ones-matmul [1,192] (contiguous
            # start/stop groups), accumulated on DVE into SBUF.
            acc = small.tile([1, 192], F32)
            nc.vector.memset(acc[:], 0.0)

            def emit_epi(prev):
                r0, nr, plps = prev
                ci, po = r0 // 128, r0 % 128
                pe = po + nr
                ets = etT[po:pe, ci, :]
                el = epi.tile([128, 64], BF16, tag="el")
                nc.scalar.activation(
                    el[po:pe, :], plps[po:pe, :],
                    mybir.ActivationFunctionType.Exp, bias=bias_shift[po:pe])
                pm = epi.tile([128, 64], BF16, tag="pm")
                nc.vector.tensor_mul(pm[po:pe, :], ets, plps[po:pe, :])
                spu = spup.tile([1, 192], F32, tag="spu")
                nc.tensor.matmul(spu[:, 0:64], ones[po:pe, :], el[po:pe, :],
                                 start=True, stop=True)
                nc.tensor.matmul(spu[:, 64:128], ones[po:pe, :], pm[po:pe, :],
                                 start=True, stop=True)
                nc.tensor.matmul(spu[:, 128:192], ones[po:pe, :], ets,
                                 start=True, stop=True)
                nc.vector.tensor_add(acc[:], acc[:], spu[:])

            # ---- feature pipeline: per piece, transposes feed featT; logits
            # matmuls use featT blocks stationary, xT moving.  Epilogue of
            # piece i-1 is emitted between piece i's transposes and matmuls
            # so the PE never waits on ACT/DVE.
            prev = None
            for (r0, nr), natc in zip(pieces, natcs, strict=True):
                ftc = ftp.tile([128, kc, 128], BF16, tag="ftc")
                for g in range(ngrp):
                    tp = tpsp.tile([128, TG, 128], BF16, tag="tps")
                    for j in range(TG):
                        k = g * TG + j
                        nc.tensor.transpose(
                            tp[:, j, 0:nr], natc[0:nr, k * 128:(k + 1) * 128],
                            ident[0:nr, 0:nr])
                    nc.vector.tensor_copy(ftc[:, g * TG:(g + 1) * TG, 0:nr],
                                          tp[:, :, 0:nr])

                if prev is not None:
                    emit_epi(prev)
                po = r0 % 128
                lps = lpsp.tile([128, 64], F32, tag="lp")
                for k in range(kc):
                    nc.tensor.matmul(
                        lps[po:po + nr, :], ftc[:, k, 0:nr], xT[:, k, :],
                        start=(k == 0), stop=(k == kc - 1),
                    )
                prev = (r0, nr, lps)
            emit_epi(prev)

            # ---- output
            nc.sync.dma_start(out_d[:], acc[:])

    nc.compile()
    return nc


_NC_CACHE = None


def _run(inputs, trace=False, **spmd_kwargs):
    global _NC_CACHE
    from concourse.bass_utils import run_bass_kernel_spmd

    x = np.ascontiguousarray(np.asarray(inputs["inputs"], dtype=np.float32))
    t = np.asarray(inputs["targets"], dtype=np.float32)
    f = np.asarray(inputs["features"], dtype=np.float32)
    # cid is unused by the reference computation.

    if _NC_CACHE is None:
        _NC_CACHE = build_nc(debug=False)
    nc = _NC_CACHE

    in_maps = []
    for c in range(NUM_CORES):
        in_maps.append({
            "inputs": x,
            "targets": np.ascontiguousarray(t[:, c * NSH:(c + 1) * NSH]),
            "features": np.ascontiguousarray(f[c * NSH:(c + 1) * NSH, :]),
        })

    res = run_bass_kernel_spmd(
        nc, in_maps, core_ids=list(range(NUM_CORES)), trace=trace, **spmd_kwargs)
    outs = np.stack([r["out"] for r in res.results])  # [8, 1, 192]

    outs64 = outs.astype(np.float64).reshape(NUM_CORES, 192)
    s = outs64[:, 0:64].sum(0)
    p = outs64[:, 64:128].sum(0)
    u = outs64[:, 128:192].sum(0)
    lse = SHIFT + np.log(s)
    loss = np.mean(lse - p / u)
    return np.float32(loss), res


def kernel(**inputs: np.ndarray) -> np.ndarray:
    loss, _ = _run(inputs)
    return np.asarray(loss, dtype=np.float32)


# revision 27
# speedup vs baseline: 1.0837x; 1.0338x over previous
"""Trainium2 Bass kernel for nn_CamMemory (soft cross-entropy vs. memory bank).

Computes: x = normalize(inputs); logits = x @ features.T / TEMP;
loss = mean_b( lse(logits_b) - dot(softmax(targets_b), logits_b) )

Sharding: features/targets split row-wise (N dim) across 8 cores; inputs
replicated.  Each core returns partial stats (s, p, u) per batch row:
  s = sum_n exp(logits - SHIFT)      (partial sum-exp, fixed shift; |logits|<=21)
  p = sum_n exp(targets - 1)*logits  (partial weighted logit sum)
  u = sum_n exp(targets - 1)         (partial softmax denominator; targets in [0,1))
Host combines: loss = mean_b( SHIFT + log(sum s) - (sum p)/(sum u) ).

Per-core schedule (wire = 16.8MB SWDGE cast-DMA of features at HBM rate,
~45us; everything else hides under it):
  - inputs/targets ride the two HWDGE rings (sync/scalar) as f32 during the
    SWDGE spin-up dead time; x-norm via ACT Square+Sqrt, scale+cast on ACT.
  - 17 feature pieces (15x128 + 2x64 rows; short tail pieces halve the
    drain): PE transpose-mode 128x128 blocks -> PSUM, DVE copies to SBUF
    featT; matmuls use featT blocks STATIONARY, xT (64 cols) MOVING ->
    logitsT [128n, 64b].  PE work for piece i-1's matmuls interleaves with
    piece i's transposes (one-piece software pipeline) so the PE never
    waits on the DVE copy it just enabled.
  - Epilogue (two-piece lag): ACT exp -> el, DVE etT*logits -> pm; s/p/u
    reduced over n by ones-matmuls ACCUMULATED in one PSUM bank across all
    pieces (disjoint 64-col ranges; per-element has_written semantics).
"""

import numpy as np

import concourse.bacc as bacc
import concourse.mybir as mybir
import concourse.tile as tile
from concourse.masks import make_identity

B = 64
D = 2048
N = 16384
NUM_CORES = 8
NSH = N // NUM_CORES  # 2048 rows of features per core
TEMP = 0.05
SHIFT = 21.0  # |logits| <= (1/TEMP)*|x.f| <= 20*(1+eps) since both unit-norm

F32 = mybir.dt.float32
BF16 = mybir.dt.bfloat16


def build_nc(d=D, nsh=NSH, b=B, debug=False):
    """Build the single-core Bass program (SPMD: same program, 8 shards)."""
    kc = d // 128     # contraction chunks (d on partitions)
    nch = nsh // 128  # feature-row chunks
    TG = 8            # transposed blocks staged per PSUM bank
    ngrp = kc // TG
    NWARM = 32

    nc = bacc.Bacc("TRN2", target_bir_lowering=False, debug=debug)

    inputs_d = nc.dram_tensor("inputs", [b, d], F32, kind="ExternalInput")
    targets_d = nc.dram_tensor("targets", [b, nsh], F32, kind="ExternalInput")
    features_d = nc.dram_tensor("features", [nsh, d], F32, kind="ExternalInput")
    out_d = nc.dram_tensor("out", [1, 192], F32, kind="ExternalOutput")

    with tile.TileContext(nc) as tc:
        with (
            tc.tile_pool(name="small", bufs=1) as small,
            tc.tile_pool(name="nat", bufs=8) as natp,
            tc.tile_pool(name="ft", bufs=4) as ftp,
            tc.tile_pool(name="epi", bufs=3) as epi,
            tc.tile_pool(name="tps", bufs=3, space="PSUM") as tpsp,
            tc.tile_pool(name="lps", bufs=3, space="PSUM") as lpsp,
            tc.tile_pool(name="spu", bufs=1, space="PSUM") as spup,
        ):
            # ---- x / targets on the HWDGE rings (parallel to SWDGE spin-up)
            xin = small.tile([b, d], F32)
            nc.sync.dma_start(xin[:], inputs_d[:])
            tg = small.tile([b, nsh], F32)
            nc.scalar.dma_start(tg[:], targets_d[:])

            # ---- feature cast-DMAs: gpsimd issues these first.
            pieces = [(i * 128, 128) for i in range(nch - 1)]
            pieces += [((nch - 1) * 128, 64), ((nch - 1) * 128 + 64, 64)]
            natcs = []
            ident = identf = None
            for i, (r0, nr) in enumerate(pieces):
                natc = natp.tile([128, d], BF16, tag="nat")
                nc.gpsimd.dma_start(natc[0:nr, :], features_d[r0:r0 + nr, :])
                natcs.append(natc)
                if i == 1:
                    ident = small.tile([128, 128], BF16)
                    make_identity(nc, ident[:])
                elif i == 3:
                    identf = small.tile([b, b], F32)
                    make_identity(nc, identf[:])

            # constants (DVE memsets; do not touch gpsimd)
            bias_m1 = small.tile([128, 1], F32)
            nc.vector.memset(bias_m1[:], -1.0)
            bias_shift = small.tile([128, 1], F32)
            nc.vector.memset(bias_shift[:], -float(SHIFT))
            ones = small.tile([128, 1], BF16)
            nc.vector.memset(ones[:], 1.0)
            wones = small.tile([128, 64], BF16)
            nc.vector.memset(wones[:], 1.0)

            # HAM pre-warm: throwaway matmuls while the first cast-DMAs are
            # in flight, so the PE clock gate is 8/8 for the real work.
            dwarm = lpsp.tile([128, 64], F32, tag="lp")
            for _ in range(NWARM):
                nc.tensor.matmul(dwarm[0:64, :], wones[:, 0:64], wones[:],
                                 start=True, stop=True)

            # ---- x norm chain (ACT-heavy; latency hides under DMA spin-up):
            # ss = sum x^2 (ACT Square+accum), srt = sqrt(T^2 ss), inv (DVE),
            # xb2 = bf16(x * inv) on ACT.
            sq = small.tile([b, d], F32)
            ss = small.tile([b, 1], F32)
            nc.scalar.activation(
                sq[:], xin[:], mybir.ActivationFunctionType.Square,
                accum_out=ss[:])
            srt = small.tile([b, 1], F32)
            nc.scalar.activation(
                srt[:], ss[:], mybir.ActivationFunctionType.Sqrt,
                scale=float(TEMP) * float(TEMP))
            inv = small.tile([b, 1], F32)
            nc.vector.reciprocal(inv[:], srt[:])
            xb2 = small.tile([b, d], BF16)
            nc.scalar.mul(xb2[:], xin[:], inv[:])

            xT = small.tile([128, kc, 64], BF16)
            etT = small.tile([128, nch, b], BF16)

            def emit_xt():
                for g in range(ngrp):
                    tpx = tpsp.tile([128, TG, 128], BF16, tag="tps")
                    for j in range(TG):
                        k = g * TG + j
                        nc.tensor.transpose(
                            tpx[:, j, 0:b], xb2[:, k * 128:(k + 1) * 128],
                            ident[0:b, 0:b])
                    nc.vector.tensor_copy(xT[:, g * TG:(g + 1) * TG, :],
                                          tpx[:, :, 0:b])

            def emit_tt():
                # f32 transposes straight from tg; exp(t-1) fuses the
                # PSUM->SBUF move on ACT (no bf16 cast pass needed).
                for g in range(nch // TG):
                    tpt = tpsp.tile([128, TG, 64], F32, tag="tps")
                    for j in range(TG):
                        c = g * TG + j
                        nc.tensor.transpose(
                            tpt[:, j, :], tg[:, c * 128:(c + 1) * 128],
                            identf[:])
                    nc.scalar.activation(
                        etT[:, g * TG:(g + 1) * TG, :], tpt[:],
                        mybir.ActivationFunctionType.Exp, bias=bias_m1[:])

            # ---- s/p/u: ones-matmuls accumulate across pieces into one
            # PSUM bank (3 disjoint col ranges -> 3 groups; per-element
            # has_written makes the interleaving safe).
            spu = spup.tile([1, 192], F32)

            def emit_epi(prev, first, last):
                r0, nr, plps = prev
                ci, po = r0 // 128, r0 % 128
                pe = po + nr
                ets = etT[po:pe, ci, :]
                el = epi.tile([128, 64], BF16, tag="el")
                nc.scalar.activation(
                    el[po:pe, :], plps[po:pe, :],
                    mybir.ActivationFunctionType.Exp, bias=bias_shift[po:pe])
                pm = epi.tile([128, 64], BF16, tag="pm")
                nc.vector.tensor_mul(pm[po:pe, :], ets, plps[po:pe, :])
                nc.tensor.matmul(spu[:, 0:64], ones[po:pe, :], el[po:pe, :],
                                 start=first, stop=last, skip_group_check=True)
                nc.tensor.matmul(spu[:, 64:128], ones[po:pe, :], pm[po:pe, :],
                                 start=first, stop=last, skip_group_check=True)
                nc.tensor.matmul(spu[:, 128:192], ones[po:pe, :], ets,
                                 start=first, stop=last, skip_group_check=True)

            def emit_mm(prev):
                r0, nr, ftc = prev
                po = r0 % 128
                lps = lpsp.tile([128, 64], F32, tag="lp")
                for k in range(kc):
                    nc.tensor.matmul(
                        lps[po:po + nr, :], ftc[:, k, 0:nr], xT[:, k, :],
                        start=(k == 0), stop=(k == kc - 1),
                    )
                return (r0, nr, lps)

            # ---- feature pipeline with one-piece matmul lag and two-piece
            # epilogue lag: PE order per piece i is
            #   transposes(i) | matmuls(i-1) | epilogue(i-2)
            # so matmuls consume copies that completed during the previous
            # piece's transposes, and epilogues consume ACT/DVE results that
            # completed during the previous piece's matmuls.
            np_ = len(pieces)
            mmq = None
            epq = None
            for idx, ((r0, nr), natc) in enumerate(
                    zip(pieces, natcs, strict=True)):
                ftc = ftp.tile([128, kc, 128], BF16, tag="ftc")
                for g in range(ngrp):
                    tp = tpsp.tile([128, TG, 128], BF16, tag="tps")
                    for j in range(TG):
                        k = g * TG + j
                        nc.tensor.transpose(
                            tp[:, j, 0:nr], natc[0:nr, k * 128:(k + 1) * 128],
                            ident[0:nr, 0:nr])
                    nc.vector.tensor_copy(ftc[:, g * TG:(g + 1) * TG, 0:nr],
                                          tp[:, :, 0:nr])
                if idx == 0:
                    emit_xt()
                elif idx == 1:
                    emit_tt()

                if epq is not None:
                    emit_epi(epq, first=(idx == 2), last=False)
                epq = None
                if mmq is not None:
                    epq = emit_mm(mmq)
                mmq = (r0, nr, ftc)
            epq2 = emit_mm(mmq)
            emit_epi(epq, first=False, last=False)
            emit_epi(epq2, first=False, last=True)

            # ---- output
            sbout = small.tile([1, 192], F32)
            nc.vector.tensor_copy(sbout[:], spu[:])
            nc.sync.dma_start(out_d[:], sbout[:])

    nc.compile()
    return nc


_NC_CACHE = None


def _run(inputs, trace=False, **spmd_kwargs):
    global _NC_CACHE
    from concourse.bass_utils import run_bass_kernel_spmd

    x = np.ascontiguousarray(np.asarray(inputs["inputs"], dtype=np.float32))
    t = np.asarray(inputs["targets"], dtype=np.float32)
    f = np.asarray(inputs["features"], dtype=np.float32)
    # cid is unused by the reference computation.

    if _NC_CACHE is None:
        _NC_CACHE = build_nc(debug=False)
    nc = _NC_CACHE

    in_maps = []
    for c in range(NUM_CORES):
        in_maps.append({
            "inputs": x,
            "targets": np.ascontiguousarray(t[:, c * NSH:(c + 1) * NSH]),
            "features": np.ascontiguousarray(f[c * NSH:(c + 1) * NSH, :]),
        })

    res = run_bass_kernel_spmd(
        nc, in_maps, core_ids=list(range(NUM_CORES)), trace=trace, **spmd_kwargs)
    outs = np.stack([r["out"] for r in res.results])  # [8, 1, 192]

    outs64 = outs.astype(np.float64).reshape(NUM_CORES, 192)
    s = outs64[:, 0:64].sum(0)
    p = outs64[:, 64:128].sum(0)
    u = outs64[:, 128:192].sum(0)
    lse = SHIFT + np.log(s)
    loss = np.mean(lse - p / u)
    return np.float32(loss), res


def kernel(**inputs: np.ndarray) -> np.ndarray:
    loss, _ = _run(inputs)
    return np.asarray(loss, dtype=np.float32)


# revision 29
# speedup vs baseline: 1.0966x; 1.0120x over previous
"""Trainium2 Bass kernel for nn_CamMemory (soft cross-entropy vs. memory bank).

Computes: x = normalize(inputs); logits = x @ features.T / TEMP;
loss = mean_b( lse(logits_b) - dot(softmax(targets_b), logits_b) )

Sharding: features/targets split row-wise (N dim) across 8 cores; inputs
replicated.  Each core returns partial stats (s, p, u) per batch row:
  s = sum_n exp(logits - SHIFT)      (partial sum-exp, fixed shift; |logits|<=21)
  p = sum_n exp(targets - 1)*logits  (partial weighted logit sum)
  u = sum_n exp(targets - 1)         (partial softmax denominator; targets in [0,1))
Host combines: loss = mean_b( SHIFT + log(sum s) - (sum p)/(sum u) ).

Per-core schedule (wire = 16.8MB SWDGE cast-DMA of features at HBM rate,
~45us; everything else hides under it):
  - inputs/targets ride the two HWDGE rings (sync/scalar) as f32 during the
    SWDGE spin-up dead time; x-norm via ACT Square+Sqrt, scale+cast on ACT.
  - 17 feature pieces (15x128 + 2x64 rows; short tail pieces halve the
    drain): PE transpose-mode 128x128 blocks -> PSUM, DVE copies to SBUF
    featT; matmuls use featT blocks STATIONARY, xT (64 cols) MOVING ->
    logitsT [128n, 64b].  PE work for piece i-1's matmuls interleaves with
    piece i's transposes (one-piece software pipeline) so the PE never
    waits on the DVE copy it just enabled.
  - Epilogue (two-piece lag): ACT exp -> el, DVE etT*logits -> pm; s/p/u
    reduced over n by ones-matmuls ACCUMULATED in one PSUM bank across all
    pieces (disjoint 64-col ranges; per-element has_written semantics).
"""

import numpy as np

import concourse.bacc as bacc
import concourse.mybir as mybir
import concourse.tile as tile
from concourse.masks import make_identity

B = 64
D = 2048
N = 16384
NUM_CORES = 8
NSH = N // NUM_CORES  # 2048 rows of features per core
TEMP = 0.05
SHIFT = 21.0  # |logits| <= (1/TEMP)*|x.f| <= 20*(1+eps) since both unit-norm

F32 = mybir.dt.float32
BF16 = mybir.dt.bfloat16


def build_nc(d=D, nsh=NSH, b=B, debug=False):
    """Build the single-core Bass program (SPMD: same program, 8 shards)."""
    kc = d // 128     # contraction chunks (d on partitions)
    nch = nsh // 128  # feature-row chunks
    TG = 8            # transposed blocks staged per PSUM bank
    ngrp = kc // TG
    NWARM = 32

    nc = bacc.Bacc("TRN2", target_bir_lowering=False, debug=debug)

    inputs_d = nc.dram_tensor("inputs", [b, d], F32, kind="ExternalInput")
    targets_d = nc.dram_tensor("targets", [b, nsh], F32, kind="ExternalInput")
    features_d = nc.dram_tensor("features", [nsh, d], F32, kind="ExternalInput")
    out_d = nc.dram_tensor("out", [1, 192], F32, kind="ExternalOutput")

    with tile.TileContext(nc) as tc:
        with (
            tc.tile_pool(name="small", bufs=1) as small,
            tc.tile_pool(name="nat", bufs=8) as natp,
            tc.tile_pool(name="ft", bufs=4) as ftp,
            tc.tile_pool(name="epi", bufs=3) as epi,
            tc.tile_pool(name="tps", bufs=3, space="PSUM") as tpsp,
            tc.tile_pool(name="lps", bufs=3, space="PSUM") as lpsp,
            tc.tile_pool(name="spu", bufs=1, space="PSUM") as spup,
        ):
            # ---- x / targets on the HWDGE rings (parallel to SWDGE spin-up)
            xin = small.tile([b, d], F32)
            nc.sync.dma_start(xin[:], inputs_d[:])
            tg = small.tile([b, nsh], F32)
            nc.sync.dma_start(tg[:], targets_d[:])

            # ---- feature cast-DMAs: gpsimd issues these first.
            pieces = [(i * 128, 128) for i in range(nch - 1)]
            pieces += [((nch - 1) * 128, 64), ((nch - 1) * 128 + 64, 64)]
            natcs = []
            ident = identf = None
            for i, (r0, nr) in enumerate(pieces):
                natc = natp.tile([128, d], BF16, tag="nat")
                nc.gpsimd.dma_start(natc[0:nr, :], features_d[r0:r0 + nr, :])
                natcs.append(natc)
                if i == 1:
                    ident = small.tile([128, 128], BF16)
                    make_identity(nc, ident[:])
                elif i == 3:
                    identf = small.tile([b, b], F32)
                    make_identity(nc, identf[:])

            # constants (DVE memsets; do not touch gpsimd)
            bias_m1 = small.tile([128, 1], F32)
            nc.vector.memset(bias_m1[:], -1.0)
            bias_shift = small.tile([128, 1], F32)
            nc.vector.memset(bias_shift[:], -float(SHIFT))
            ones = small.tile([128, 1], BF16)
            nc.vector.memset(ones[:], 1.0)
            wones = small.tile([128, 64], BF16)
            nc.vector.memset(wones[:], 1.0)

            # HAM pre-warm: throwaway matmuls while the first cast-DMAs are
            # in flight, so the PE clock gate is 8/8 for the real work.
            dwarm = lpsp.tile([128, 64], F32, tag="lp")
            for _ in range(NWARM):
                nc.tensor.matmul(dwarm[0:64, :], wones[:, 0:64], wones[:],
                                 start=True, stop=True)

            # ---- x norm chain (ACT-heavy; latency hides under DMA spin-up):
            # ss = sum x^2 (ACT Square+accum), srt = sqrt(T^2 ss), inv (DVE),
            # xb2 = bf16(x * inv) on ACT.
            sq = small.tile([b, d], F32)
            ss = small.tile([b, 1], F32)
            nc.scalar.activation(
                sq[:], xin[:], mybir.ActivationFunctionType.Square,
                accum_out=ss[:])
            srt = small.tile([b, 1], F32)
            nc.scalar.activation(
                srt[:], ss[:], mybir.ActivationFunctionType.Sqrt,
                scale=float(TEMP) * float(TEMP))
            inv = small.tile([b, 1], F32)
            nc.vector.reciprocal(inv[:], srt[:])
            xb2 = small.tile([b, d], BF16)
            nc.vector.tensor_scalar_mul(xb2[:], xin[:], inv[:])

            xT = small.tile([128, kc, 64], BF16)
            etT = small.tile([128, nch, b], BF16)

            def emit_xt():
                for g in range(ngrp):
                    tpx = tpsp.tile([128, TG, 128], BF16, tag="tps")
                    for j in range(TG):
                        k = g * TG + j
                        nc.tensor.transpose(
                            tpx[:, j, 0:b], xb2[:, k * 128:(k + 1) * 128],
                            ident[0:b, 0:b])
                    nc.vector.tensor_copy(xT[:, g * TG:(g + 1) * TG, :],
                                          tpx[:, :, 0:b])

            def emit_tt():
                # f32 transposes straight from tg; exp(t-1) fuses the
                # PSUM->SBUF move on ACT (no bf16 cast pass needed).
                for g in range(nch // TG):
                    tpt = tpsp.tile([128, TG, 64], F32, tag="tps")
                    for j in range(TG):
                        c = g * TG + j
                        nc.tensor.transpose(
                            tpt[:, j, :], tg[:, c * 128:(c + 1) * 128],
                            identf[:])
                    nc.scalar.activation(
                        etT[:, g * TG:(g + 1) * TG, :], tpt[:],
                        mybir.ActivationFunctionType.Exp, bias=bias_m1[:])

            # ---- s/p/u: ones-matmuls accumulate across pieces into one
            # PSUM bank (3 disjoint col ranges -> 3 groups; per-element
            # has_written makes the interleaving safe).
            spu = spup.tile([1, 192], F32)

            def emit_epi(prev, first, last):
                r0, nr, plps = prev
                ci, po = r0 // 128, r0 % 128
                pe = po + nr
                ets = etT[po:pe, ci, :]
                el = epi.tile([128, 64], BF16, tag="el")
                nc.scalar.activation(
                    el[po:pe, :], plps[po:pe, :],
                    mybir.ActivationFunctionType.Exp, bias=bias_shift[po:pe])
                pm = epi.tile([128, 64], BF16, tag="pm")
                nc.vector.tensor_mul(pm[po:pe, :], ets, plps[po:pe, :])
                nc.tensor.matmul(spu[:, 0:64], ones[po:pe, :], el[po:pe, :],
                                 start=first, stop=last, skip_group_check=True)
                nc.tensor.matmul(spu[:, 64:128], ones[po:pe, :], pm[po:pe, :],
                                 start=first, stop=last, skip_group_check=True)
                nc.tensor.matmul(spu[:, 128:192], ones[po:pe, :], ets,
                                 start=first, stop=last, skip_group_check=True)

            def emit_mm(prev):
                r0, nr, ftc = prev
                po = r0 % 128
                lps = lpsp.tile([128, 64], F32, tag="lp")
                for k in range(kc):
                    nc.tensor.matmul(
                        lps[po:po + nr, :], ftc[:, k, 0:nr], xT[:, k, :],
                        start=(k == 0), stop=(k == kc - 1),
                    )
                return (r0, nr, lps)

            # ---- feature pipeline with one-piece matmul lag and two-piece
            # epilogue lag: PE order per piece i is
            #   transposes(i) | matmuls(i-1) | epilogue(i-2)
            # so matmuls consume copies that completed during the previous
            # piece's transposes, and epilogues consume ACT/DVE results that
            # completed during the previous piece's matmuls.
            np_ = len(pieces)
            mmq = None
            epq = None
            for idx, ((r0, nr), natc) in enumerate(
                    zip(pieces, natcs, strict=True)):
                ftc = ftp.tile([128, kc, 128], BF16, tag="ftc")
                for g in range(ngrp):
                    tp = tpsp.tile([128, TG, 128], BF16, tag="tps")
                    for j in range(TG):
                        k = g * TG + j
                        nc.tensor.transpose(
                            tp[:, j, 0:nr], natc[0:nr, k * 128:(k + 1) * 128],
                            ident[0:nr, 0:nr])
                    nc.vector.tensor_copy(ftc[:, g * TG:(g + 1) * TG, 0:nr],
                                          tp[:, :, 0:nr])
                if idx == 0:
                    emit_xt()
                elif idx == 1:
                    emit_tt()

                if epq is not None:
                    emit_epi(epq, first=(idx == 2), last=False)
                epq = None
                if mmq is not None:
                    epq = emit_mm(mmq)
                mmq = (r0, nr, ftc)
            epq2 = emit_mm(mmq)
            emit_epi(epq, first=False, last=False)
            emit_epi(epq2, first=False, last=True)

            # ---- output
            sbout = small.tile([1, 192], F32)
            nc.vector.tensor_copy(sbout[:], spu[:])
            nc.sync.dma_start(out_d[:], sbout[:])

    nc.compile()
    return nc


_NC_CACHE = None


def _run(inputs, trace=False, **spmd_kwargs):
    global _NC_CACHE
    from concourse.bass_utils import run_bass_kernel_spmd

    x = np.ascontiguousarray(np.asarray(inputs["inputs"], dtype=np.float32))
    t = np.asarray(inputs["targets"], dtype=np.float32)
    f = np.asarray(inputs["features"], dtype=np.float32)
    # cid is unused by the reference computation.

    if _NC_CACHE is None:
        _NC_CACHE = build_nc(debug=False)
    nc = _NC_CACHE

    in_maps = []
    for c in range(NUM_CORES):
        in_maps.append({
            "inputs": x,
            "targets": np.ascontiguousarray(t[:, c * NSH:(c + 1) * NSH]),
            "features": np.ascontiguousarray(f[c * NSH:(c + 1) * NSH, :]),
        })

    res = run_bass_kernel_spmd(
        nc, in_maps, core_ids=list(range(NUM_CORES)), trace=trace, **spmd_kwargs)
    outs = np.stack([r["out"] for r in res.results])  # [8, 1, 192]

    outs64 = outs.astype(np.float64).reshape(NUM_CORES, 192)
    s = outs64[:, 0:64].sum(0)
    p = outs64[:, 64:128].sum(0)
    u = outs64[:, 128:192].sum(0)
    lse = SHIFT + np.log(s)
    loss = np.mean(lse - p / u)
    return np.float32(loss), res


def kernel(**inputs: np.ndarray) -> np.ndarray:
    loss, _ = _run(inputs)
    return np.asarray(loss, dtype=np.float32)
